# revision 1
# baseline (speedup 1.0000x reference)
"""Multi-head attention (B=2, S=4096, D=768, H=12, hd=64) on 8 trn2 NeuronCores.

Sharding: core c -> batch b = c//4, heads [3*(c%4), 3*(c%4)+3)  (batch- and
head-parallel; no device collectives).  Each core computes the partial
output  sum_h softmax((x Wq_h + bq_h)(x Wk_h + bk_h)^T / 8) (x Wv_h) Wo_h
for its 3 heads as a full [S, 768] f32 tensor; the host sums the 4 partials
per batch and adds the bias terms (bo + bv @ Wo, since softmax rows sum to 1).

Per-core device algorithm (all matmuls bf16, f32 psum accumulate):
  - host ships x[b]^T as [6,128,S] (d-major), weights packed per head group
  - qT/kT projections -> [d, s]-layout tiles; V projection -> [s, d] layout
    with a ones column appended
  - scores computed transposed: ST[k-block, q-chunk] = kT^T q, exp on ACT
    (no max subtraction: |scores/8| <~ 2 for this problem)
  - attn_out^T accumulated directly: acc[0:64, q] = sum_k V[k,:]^T P^T[k, q],
    row 64 = sum_k exp (the ones column) -> lhsT is V (65 cols, weight load
    hidden under the 512-wide stream), and no output transpose is needed
  - normalized with a reciprocal of row 64, partition-broadcast via a DRAM
    round trip, then the final projection against Wo rows
"""

import numpy as np
from contextlib import ExitStack

import concourse.bass as bass
import concourse.bacc as bacc
import concourse.mybir as mybir
from concourse import tile

BF16 = mybir.dt.bfloat16
F32 = mybir.dt.float32
AF = mybir.ActivationFunctionType

D_MODEL = 768
N_HEADS = 12
HD = 64
N_CORES = 8
NH_LOC = 3          # heads per core
DC = D_MODEL // 128  # 6 chunks of d_model
CHUNK = 512          # q columns processed per score chunk
GRP = 3              # k-blocks (of 128) per psum score tile / exp call


def build(nc, S, level=3):
    """Emit the per-core program (SPMD; all cores run this with their shard).

    level: debug knob — 1 = projections only, 2 = + attention, 3 = full.
    """
    SB = S // 128     # seq blocks of 128
    NCH = S // CHUNK  # q chunks
    KB = S // 128     # k blocks of 128

    xT_d = nc.declare_dram_parameter("xT", [DC, 128, S], BF16, isOutput=False)
    wqk_d = nc.declare_dram_parameter("wqk", [3, DC, 128, 128], BF16, isOutput=False)
    bqk_d = nc.declare_dram_parameter("bqk", [128, 3], F32, isOutput=False)
    wv_d = nc.declare_dram_parameter("wv", [DC, 128, NH_LOC * HD], BF16, isOutput=False)
    wo_d = nc.declare_dram_parameter("wo", [3, 128, D_MODEL], BF16, isOutput=False)
    out_d = nc.declare_dram_parameter("out", [S, D_MODEL], F32, isOutput=True)

    with tile.TileContext(nc) as tc, ExitStack() as ctx:
        const = ctx.enter_context(tc.tile_pool(name="const", bufs=1))

        def ctile(name, shape, dt):
            return const.tile(shape, dt, tag=name, name=name)

        # --- constants / long-lived tensors -------------------------------
        XH = S // 2 if S >= 1024 else S   # xT column-half size
        xts = [ctile(f"xt{i}", [128, XH], BF16)
               for i in range(DC * (S // XH))]

        def xth(dcc, off, ln):
            # slice [off, off+ln) of logical xT chunk dcc (ln divides XH)
            t = xts[dcc * (S // XH) + off // XH]
            lo = off % XH
            return t[:, lo:lo + ln]
        wqks = [ctile(f"wqk{i}", [128, DC * 128], BF16) for i in range(3)]
        bqks = ctile("bqk", [128, 3], F32)
        wvs = [ctile(f"wv{i}", [128, NH_LOC * HD], BF16) for i in range(DC)]
        wos = [ctile(f"wo{i}", [128, D_MODEL], BF16) for i in range(NH_LOC)]
        v1s = [ctile(f"v1_{h}", [128, 65 * KB], BF16) for h in range(NH_LOC)]
        qts = [ctile(f"qt{i}", [128, S], BF16) for i in range(NH_LOC)]
        kts = [ctile(f"kt{i}", [128, S], BF16) for i in range(NH_LOC)]
        ats = [[ctile(f"at{i}_{qc}", [128, CHUNK], BF16)
                for qc in range(NCH)] for i in range(NH_LOC)]

        pt_pool = ctx.enter_context(tc.tile_pool(name="pt", bufs=10))
        outst_pool = ctx.enter_context(tc.tile_pool(name="outst", bufs=2))
        small_pool = ctx.enter_context(tc.tile_pool(name="small", bufs=2))
        rb_pool = ctx.enter_context(tc.tile_pool(name="rb", bufs=2))
        dram_pool = ctx.enter_context(tc.tile_pool(name="drs", bufs=3, space="DRAM"))
        # ONE psum pool layout for the whole kernel (no pool releases -> no
        # cross-phase serialization): 6 banks of score tiles + 2 banks shared
        # (same tag) by projection / P@V-accumulator / final-projection tiles.
        ps_st = ctx.enter_context(tc.tile_pool(name="ps_st", bufs=2, space="PSUM"))
        ps_sh = ctx.enter_context(tc.tile_pool(name="ps_sh", bufs=2, space="PSUM"))

        def shtile(nm):
            return ps_sh.tile([128, 512], F32, tag="ps", name=nm)

        # --- load inputs ---------------------------------------------------
        # first halves of xT + q/k weights first: the first projection
        # units depend only on these, so the PE starts ~7us earlier
        for i in range(DC):
            nc.sync.dma_start(xts[i * (S // XH)][:], xT_d[i, :, 0:XH])
        for blk in range(2):
            for dcc in range(DC):
                nc.sync.dma_start(
                    wqks[blk][:, dcc * 128:(dcc + 1) * 128], wqk_d[blk, dcc]
                )
        nc.sync.dma_start(bqks[:], bqk_d[:])
        for i in range(DC):
            for hh in range(1, S // XH):
                nc.sync.dma_start(xts[i * (S // XH) + hh][:],
                                  xT_d[i, :, hh * XH:(hh + 1) * XH])
        for dcc in range(DC):
            nc.sync.dma_start(
                wqks[2][:, dcc * 128:(dcc + 1) * 128], wqk_d[2, dcc]
            )
        for i in range(DC):
            nc.sync.dma_start(wvs[i][:], wv_d[i])
        for i in range(NH_LOC):
            nc.sync.dma_start(wos[i][:], wo_d[i])
        for h in range(NH_LOC):
            nc.gpsimd.memset(v1s[h][:], 1.0)
        # zero halves: q/k rows carrying the contraction zero-padding that
        # keeps every matmul at K=128 (K=64 matmuls never warm the PE HAM
        # clock gate and run at half clock)
        for (t, z0, z1) in [(qts[0], 64, 128), (qts[1], 0, 64),
                            (qts[2], 64, 128), (kts[0], 64, 128),
                            (kts[1], 0, 64), (kts[2], 64, 128)]:
            nc.gpsimd.memset(t[z0:z1, :], 0.0)
        for h in range(NH_LOC):
            for qc in range(NCH):
                nc.gpsimd.memset(ats[h][qc][HD:128, :], 0.0)

        # --- phase 1: projections -----------------------------------------
        def proj_qk(blk):
            # qT / kT block: [d_out(128 part), s] = W_blk^T x^T
            # blk0 = [q0 q1] -> Q0 rows 0:64 / Q1 rows 64:128
            # blk1 = [k0 k1] -> K0 / K1
            # blk2 = [q2 k2] -> Q2 rows 0:64; k2 half is bias-added into a
            #   staging tile (same partitions 64:128) then DMA-moved to K2
            #   rows 0:64 (only DMA can shift partitions)
            for sc in range(S // 512):
                pp = shtile(f"pp{blk}_{sc}")
                for dcc in range(DC):
                    nc.tensor.matmul(
                        pp[:],
                        lhsT=wqks[blk][:, dcc * 128:(dcc + 1) * 128],
                        rhs=xth(dcc, sc * 512, 512),
                        start=(dcc == 0),
                        stop=(dcc == DC - 1),
                    )
                sl = slice(sc * 512, (sc + 1) * 512)
                if blk == 0 or blk == 1:
                    dsts = qts if blk == 0 else kts
                    nc.vector.tensor_scalar_add(
                        dsts[0][0:64, sl], pp[0:64, :], bqks[0:64, blk:blk + 1])
                    nc.vector.tensor_scalar_add(
                        dsts[1][64:128, sl], pp[64:128, :], bqks[64:128, blk:blk + 1])
                else:
                    nc.vector.tensor_scalar_add(
                        qts[2][0:64, sl], pp[0:64, :], bqks[0:64, 2:3])
                    k2s = small_pool.tile([128, 512], BF16, tag="k2s",
                                          name=f"k2s{sc}")
                    nc.vector.tensor_scalar_add(
                        k2s[64:128, :], pp[64:128, :], bqks[64:128, 2:3])
                    nc.sync.dma_start(kts[2][0:64, sl], k2s[64:128, :])

        def proj_v():
            # V in [s, d] layout; the 65-col stride keeps the ones column
            for sb in range(SB):
                pv = shtile(f"pv{sb}")
                pvv = pv[:, 0:NH_LOC * HD]
                for dcc in range(DC):
                    nc.tensor.matmul(
                        pvv,
                        lhsT=xth(dcc, sb * 128, 128),
                        rhs=wvs[dcc][:],
                        start=(dcc == 0),
                        stop=(dcc == DC - 1),
                    )
                for h in range(NH_LOC):
                    nc.vector.tensor_copy(
                        v1s[h][:, sb * 65: sb * 65 + 64],
                        pv[:, h * HD:(h + 1) * HD],
                    )

        if level < 2:
            proj_qk(0)
            proj_qk(1)
            proj_qk(2)
            proj_v()
            for sb in range(SB):
                ost = outst_pool.tile([128, D_MODEL], F32, tag="ost",
                                      name=f"ost{sb}")
                nc.vector.memset(ost[:], 0.0)
                nc.sync.dma_start(out_d[sb * 128:(sb + 1) * 128, :], ost[:])
            return nc

        # --- phase 2+3: attention, heads interleaved per q-chunk; the
        # final projection for a chunk's s-blocks is emitted right after its
        # three heads finish, so PE always has fill work and there is no
        # serial projection tail.
        groups = []
        j0 = 0
        while j0 < KB:
            groups.append((j0, min(GRP, KB - j0)))
            j0 += GRP

        def phase_a(h, qc, g0, glen):
            qt, kt = qts[h], kts[h]
            st = ps_st.tile([128, GRP * CHUNK], F32, tag="st",
                            name=f"st{h}_{qc}_{g0}")
            for t in range(glen):
                j = g0 + t
                nc.tensor.matmul(
                    st[:, t * CHUNK:(t + 1) * CHUNK],
                    lhsT=kt[:, j * 128:(j + 1) * 128],
                    rhs=qt[:, qc * CHUNK:(qc + 1) * CHUNK],
                    start=True,
                    stop=True,
                )
            pt = pt_pool.tile([128, GRP * CHUNK], BF16, tag="pt",
                              name=f"pt{h}_{qc}_{g0}")
            nc.scalar.activation(
                pt[:, 0:glen * CHUNK],
                st[:, 0:glen * CHUNK],
                AF.Exp,
                scale=0.125,
            )
            return pt

        def fin(qc):
            # final projection for chunk qc's s-blocks (emitted one chunk
            # late so the normalize DMA round trip is off the critical path).
            # NOTE: all accumulating matmuls in one psum group must share one
            # tile_position (mixed row offsets crash the device), so every
            # head's AT / Wo tile lives at partition offset 0 with the K=128
            # zero padding.
            for sb in range(qc * (CHUNK // 128), (qc + 1) * (CHUNK // 128)):
                ost = outst_pool.tile([128, D_MODEL], F32, tag="ost",
                                      name=f"ost{sb}")
                for (n0, n1) in ((0, 512), (512, D_MODEL)):
                    po = shtile(f"fp{sb}_{n0}")
                    pon = po[:, 0:n1 - n0]
                    sb_in = sb % (CHUNK // 128)
                    for h in range(NH_LOC):
                        nc.tensor.matmul(
                            pon,
                            lhsT=ats[h][qc][:, sb_in * 128:(sb_in + 1) * 128],
                            rhs=wos[h][:, n0:n1],
                            start=(h == 0),
                            stop=(h == NH_LOC - 1),
                        )
                    nc.vector.tensor_copy(ost[:, n0:n1], pon)
                nc.gpsimd.dma_start(out_d[sb * 128:(sb + 1) * 128, :], ost[:])

        proj_qk(0)
        proj_qk(1)
        # hoist (qc=0, h=0) scores+exp ahead of the V projection: ACT starts
        # as soon as q0/k0 exist; the matching P@V accumulation (which needs
        # V) is emitted in the main loop below and the scheduler orders it
        # after proj_v via the v1s dependency.
        pts0 = [phase_a(0, 0, g0, glen) for (g0, glen) in groups]
        proj_v()

        for qc in range(NCH):
            for h in range(NH_LOC):
                if qc == 0 and h == 2:
                    proj_qk(2)
                if level >= 3 and qc > 0 and h == 1:
                    fin(qc - 1)
                # acc[d, q] = sum_k [V|1][k,:]^T exp(ST)[k, q]:
                # rows 0..63 = attn_out^T (unnormalized), row 64 = sum(exp)
                acc = shtile(f"acc{h}_{qc}")
                for gi, (g0, glen) in enumerate(groups):
                    if qc == 0 and h == 0:
                        pt = pts0[gi]
                    else:
                        pt = phase_a(h, qc, g0, glen)
                    for t in range(glen):
                        j = g0 + t
                        nc.tensor.matmul(
                            acc[0:65, :],
                            lhsT=v1s[h][:, j * 65:(j + 1) * 65],
                            rhs=pt[:, t * CHUNK:(t + 1) * CHUNK],
                            start=(j == 0),
                            stop=(j == KB - 1),
                        )
                # normalize: copy the accumulator off psum (frees the shared
                # slot), broadcast the sum row to partitions 0..63 via a DRAM
                # round trip (DMA can't read PSUM; SBUF APs can't have a zero
                # partition step; DVE can't shift partitions).
                tmp = small_pool.tile([65, CHUNK], F32, tag="r1",
                                      name=f"r1_{h}_{qc}")
                nc.vector.tensor_copy(tmp[:], acc[0:65, :])
                drs = dram_pool.tile([1, CHUNK], F32, tag="drs",
                                     name=f"drs{h}_{qc}")
                nc.sync.dma_start(drs[:], tmp[64:65, :])
                rbs = rb_pool.tile([HD, CHUNK], F32, tag="rbs",
                                   name=f"rbs{h}_{qc}")
                nc.sync.dma_start(rbs[:], drs[:].to_broadcast([HD, CHUNK]))
                rbr = rb_pool.tile([HD, CHUNK], F32, tag="rbr",
                                   name=f"rbr{h}_{qc}")
                nc.vector.reciprocal(rbr[:], rbs[:])
                nc.vector.tensor_mul(
                    ats[h][qc][0:HD, :],
                    tmp[0:HD, :],
                    rbr[:],
                )

        if level < 3:
            for sb in range(SB):
                ost = outst_pool.tile([128, D_MODEL], F32, tag="ost",
                                      name=f"ost{sb}")
                nc.vector.memset(ost[:], 0.0)
                nc.sync.dma_start(out_d[sb * 128:(sb + 1) * 128, :], ost[:])
            return nc
        fin(NCH - 1)

    return nc


def make_nc(S=4096, level=3):
    nc = bacc.Bacc(None, target_bir_lowering=False, debug=False)
    build(nc, S, level=level)
    nc.compile()
    return nc


def shard_inputs(x, Wq, bq, Wk, bk, Wv, bv, Wo, bo, S):
    """Host-side packing of the 8 per-core input maps (bf16 casts included)."""
    import ml_dtypes

    bf = ml_dtypes.bfloat16
    in_maps = []
    for c in range(N_CORES):
        b = c // 4
        h0 = NH_LOC * (c % 4)
        cs, ce = h0 * HD, (h0 + NH_LOC) * HD
        xT = np.ascontiguousarray(x[b].T).astype(bf).reshape(DC, 128, S)

        def blkify(w2):  # [768, 128] -> [DC, 128, 128]
            return np.ascontiguousarray(w2).astype(bf).reshape(DC, 128, 128)

        wqk = np.stack([
            blkify(Wq[:, cs:cs + 2 * HD]),
            blkify(Wk[:, cs:cs + 2 * HD]),
            blkify(np.concatenate([Wq[:, cs + 2 * HD:ce],
                                   Wk[:, cs + 2 * HD:ce]], axis=1)),
        ])
        bqk = np.stack([
            bq[cs:cs + 2 * HD],
            bk[cs:cs + 2 * HD],
            np.concatenate([bq[cs + 2 * HD:ce], bk[cs + 2 * HD:ce]]),
        ], axis=1).astype(np.float32)  # [128, 3]
        wv = np.ascontiguousarray(Wv[:, cs:ce]).astype(bf).reshape(
            DC, 128, NH_LOC * HD)
        wo = np.zeros((NH_LOC, 128, D_MODEL), np.float32)
        wo[:, 0:HD, :] = Wo[cs:ce, :].reshape(NH_LOC, HD, D_MODEL)
        wo = wo.astype(bf)
        in_maps.append({"xT": xT, "wqk": wqk, "bqk": bqk, "wv": wv, "wo": wo})
    return in_maps


_NC_CACHE = {}


def kernel(x, Wq, bq, Wk, bk, Wv, bv, Wo, bo):
    from concourse import bass_utils

    x = np.asarray(x, np.float32)
    Wq, bq = np.asarray(Wq, np.float32), np.asarray(bq, np.float32)
    Wk, bk = np.asarray(Wk, np.float32), np.asarray(bk, np.float32)
    Wv, bv = np.asarray(Wv, np.float32), np.asarray(bv, np.float32)
    Wo, bo = np.asarray(Wo, np.float32), np.asarray(bo, np.float32)
    B, S, D = x.shape
    assert (B, D) == (2, D_MODEL)
    if S not in _NC_CACHE:
        _NC_CACHE[S] = make_nc(S)
    nc = _NC_CACHE[S]

    in_maps = shard_inputs(x, Wq, bq, Wk, bk, Wv, bv, Wo, bo, S)
    res = bass_utils.run_bass_kernel_spmd(nc, in_maps, core_ids=list(range(N_CORES)))

    # host reduction: sum head-group partials per batch, add bias terms
    bias = (bo.astype(np.float32)
            + bv.astype(np.float32) @ Wo.astype(np.float32))  # [768]
    out = np.empty((B, S, D_MODEL), np.float32)
    for b in range(B):
        acc = res.results[4 * b]["out"].astype(np.float32).copy()
        for c in range(4 * b + 1, 4 * b + 4):
            acc += res.results[c]["out"]
        out[b] = acc + bias
    return out



# revision 2
# speedup vs baseline: 1.0352x; 1.0352x over previous
"""Multi-head attention (B=2, S=4096, D=768, H=12, hd=64) on 8 trn2 NeuronCores.

Sharding: core c -> batch b = c//4, heads [3*(c%4), 3*(c%4)+3)  (batch- and
head-parallel; no device collectives).  Each core computes the partial
output  sum_h softmax((x Wq_h + bq_h)(x Wk_h + bk_h)^T / 8) (x Wv_h) Wo_h
for its 3 heads as a full [S, 768] f32 tensor; the host sums the 4 partials
per batch and adds the bias terms (bo + bv @ Wo, since softmax rows sum to 1).

Per-core device algorithm (all matmuls bf16, f32 psum accumulate):
  - host ships x[b]^T as [6,128,S] (d-major); x is DMA'd column-major in
    512-col chunks (all 6 d-blocks of a chunk together) so the first
    projection matmul can start after ~1MB of traffic instead of ~4MB
  - qT/kT projections -> [d, s]-layout tiles, emitted blk0/blk1
    interleaved per 512-chunk; each head's 64 d-rows are then REPLICATED
    to the other partition half by a small DMA (only DMA can shift
    partitions), so q/k live on partitions 0:64 and 64:128
  - scores computed transposed as K=64 matmuls: ST[k-block, q-chunk] =
    kT^T q with tile_position=(0,0) for even k-blocks (reading the lo
    half) and (64,0) for odd k-blocks (hi half).  Adjacent matmuls hit
    different PE row-groups and execute concurrently (~2x), recovering
    the utilization lost to hd=64 < 128.  exp on ACT (no max
    subtraction: |scores/8| <~ 2 for this problem)
  - attn_out^T accumulated directly: acc[0:64, q] = sum_k V[k,:]^T P^T[k, q],
    row 64 = sum_k exp (the ones column) -> lhsT is V (65 cols, weight load
    hidden under the 512-wide stream), and no output transpose is needed
  - normalized with reciprocal_approx_fast of row 64 (5x faster than the
    exact DVE reciprocal; denominators are O(1e3) so approx is safe),
    partition-broadcast via a DRAM round trip, then the final projection
    against Wo rows (K=128 with zero padding: mixed tile_position row
    offsets in one accumulation group crash the device)
"""

import numpy as np
from contextlib import ExitStack

import concourse.bass as bass
import concourse.bacc as bacc
import concourse.mybir as mybir
from concourse import tile

BF16 = mybir.dt.bfloat16
F32 = mybir.dt.float32
AF = mybir.ActivationFunctionType

D_MODEL = 768
N_HEADS = 12
HD = 64
N_CORES = 8
NH_LOC = 3          # heads per core
DC = D_MODEL // 128  # 6 chunks of d_model
CHUNK = 512          # q columns processed per score chunk
GRP = 3              # k-blocks (of 128) per psum score tile / exp call
PAIR = True          # K=64 row-tiled score matmuls (vs K=128 zero-padded)


def build(nc, S, level=3):
    """Emit the per-core program (SPMD; all cores run this with their shard).

    level: debug knob — 1 = projections only, 2 = + attention, 3 = full.
    """
    SB = S // 128     # seq blocks of 128
    NCH = S // CHUNK  # q chunks
    KB = S // 128     # k blocks of 128

    xT_d = nc.declare_dram_parameter("xT", [DC, 128, S], BF16, isOutput=False)
    wqk_d = nc.declare_dram_parameter("wqk", [3, DC, 128, 128], BF16, isOutput=False)
    bqk_d = nc.declare_dram_parameter("bqk", [128, 3], F32, isOutput=False)
    wv_d = nc.declare_dram_parameter("wv", [DC, 128, NH_LOC * HD], BF16, isOutput=False)
    wo_d = nc.declare_dram_parameter("wo", [3, 128, D_MODEL], BF16, isOutput=False)
    out_d = nc.declare_dram_parameter("out", [S, D_MODEL], F32, isOutput=True)

    with tile.TileContext(nc) as tc, ExitStack() as ctx:
        const = ctx.enter_context(tc.tile_pool(name="const", bufs=1))

        def ctile(name, shape, dt):
            return const.tile(shape, dt, tag=name, name=name)

        # --- constants / long-lived tensors -------------------------------
        XH = 512 if S >= 1024 else S     # xT column-chunk size
        NXH = S // XH
        xts = [ctile(f"xt{i}", [128, XH], BF16) for i in range(DC * NXH)]

        def xth(dcc, off, ln):
            # slice [off, off+ln) of logical xT chunk dcc (ln divides XH)
            t = xts[dcc * NXH + off // XH]
            lo = off % XH
            return t[:, lo:lo + ln]
        wqks = [ctile(f"wqk{i}", [128, DC * 128], BF16) for i in range(3)]
        bqks = ctile("bqk", [128, 3], F32)
        wvs = [ctile(f"wv{i}", [128, NH_LOC * HD], BF16) for i in range(DC)]
        wos = [ctile(f"wo{i}", [128, D_MODEL], BF16) for i in range(NH_LOC)]
        v1s = [ctile(f"v1_{h}", [128, 65 * KB], BF16) for h in range(NH_LOC)]
        qts = [ctile(f"qt{i}", [128, S], BF16) for i in range(NH_LOC)]
        kts = [ctile(f"kt{i}", [128, S], BF16) for i in range(NH_LOC)]
        ats = [[ctile(f"at{i}_{qc}", [128, CHUNK], BF16)
                for qc in range(NCH)] for i in range(NH_LOC)]

        pt_pool = ctx.enter_context(tc.tile_pool(name="pt", bufs=10))
        outst_pool = ctx.enter_context(tc.tile_pool(name="outst", bufs=2))
        small_pool = ctx.enter_context(tc.tile_pool(name="small", bufs=2))
        rb_pool = ctx.enter_context(tc.tile_pool(name="rb", bufs=2))
        dram_pool = ctx.enter_context(tc.tile_pool(name="drs", bufs=3, space="DRAM"))
        # ONE psum pool layout for the whole kernel (no pool releases -> no
        # cross-phase serialization): 6 banks of score tiles + 2 banks shared
        # (same tag) by projection / P@V-accumulator / final-projection tiles.
        ps_st = ctx.enter_context(tc.tile_pool(name="ps_st", bufs=2, space="PSUM"))
        ps_sh = ctx.enter_context(tc.tile_pool(name="ps_sh", bufs=2, space="PSUM"))

        def shtile(nm):
            return ps_sh.tile([128, 512], F32, tag="ps", name=nm)

        # --- load inputs ---------------------------------------------------
        # q/k weights + x column-chunk 0 first: the first projection units
        # depend only on these, so the PE starts within a few us
        for dcc in range(DC):
            nc.sync.dma_start(wqks[0][:, dcc * 128:(dcc + 1) * 128],
                              wqk_d[0, dcc])
        for dcc in range(DC):
            nc.sync.dma_start(xts[dcc * NXH][:], xT_d[dcc, :, 0:XH])
        for dcc in range(DC):
            nc.sync.dma_start(wqks[1][:, dcc * 128:(dcc + 1) * 128],
                              wqk_d[1, dcc])
        nc.sync.dma_start(bqks[:], bqk_d[:])
        for ch in range(1, 3):
            for dcc in range(DC):
                nc.sync.dma_start(xts[dcc * NXH + ch][:],
                                  xT_d[dcc, :, ch * XH:(ch + 1) * XH])
        for dcc in range(DC):
            nc.sync.dma_start(wqks[2][:, dcc * 128:(dcc + 1) * 128],
                              wqk_d[2, dcc])
        for dcc in range(DC):
            nc.sync.dma_start(wvs[dcc][:], wv_d[dcc])
        for ch in range(3, NXH):
            for dcc in range(DC):
                nc.sync.dma_start(xts[dcc * NXH + ch][:],
                                  xT_d[dcc, :, ch * XH:(ch + 1) * XH])
        for i in range(NH_LOC):
            nc.sync.dma_start(wos[i][:], wo_d[i])
        for h in range(NH_LOC):
            nc.gpsimd.memset(v1s[h][:], 1.0)
        if not PAIR:
            # zero halves: q/k contraction zero-padding keeps matmuls at K=128
            for (t, z0, z1) in [(qts[0], 64, 128), (qts[1], 0, 64),
                                (qts[2], 64, 128), (kts[0], 64, 128),
                                (kts[1], 0, 64), (kts[2], 64, 128)]:
                nc.gpsimd.memset(t[z0:z1, :], 0.0)
        for h in range(NH_LOC):
            for qc in range(NCH):
                nc.gpsimd.memset(ats[h][qc][HD:128, :], 0.0)

        # --- phase 1: projections -----------------------------------------
        def repl(dst, lo_src, sl):
            # replicate a 64-partition half to the other half (DMA only)
            if PAIR:
                nc.gpsimd.dma_start(dst, lo_src)

        def proj_chunk(blk, sc):
            # qT / kT block: [d_out(128 part), s] = W_blk^T x^T
            # blk0 = [q0 q1] -> Q0 rows 0:64 / Q1 rows 64:128
            # blk1 = [k0 k1] -> K0 / K1
            # blk2 = [q2 k2] -> Q2 rows 0:64; k2 rows 64:128 (bias-added in
            #   place, partitions match the psum half)
            # each 64-row half is then DMA-replicated to the other half
            pp = shtile(f"pp{blk}_{sc}")
            for dcc in range(DC):
                nc.tensor.matmul(
                    pp[:],
                    lhsT=wqks[blk][:, dcc * 128:(dcc + 1) * 128],
                    rhs=xth(dcc, sc * 512, 512),
                    start=(dcc == 0),
                    stop=(dcc == DC - 1),
                )
            sl = slice(sc * 512, (sc + 1) * 512)
            if blk == 0 or blk == 1:
                dsts = qts if blk == 0 else kts
                nc.vector.tensor_scalar_add(
                    dsts[0][0:64, sl], pp[0:64, :], bqks[0:64, blk:blk + 1])
                nc.vector.tensor_scalar_add(
                    dsts[1][64:128, sl], pp[64:128, :], bqks[64:128, blk:blk + 1])
                repl(dsts[0][64:128, sl], dsts[0][0:64, sl], sl)
                repl(dsts[1][0:64, sl], dsts[1][64:128, sl], sl)
            else:
                nc.vector.tensor_scalar_add(
                    qts[2][0:64, sl], pp[0:64, :], bqks[0:64, 2:3])
                nc.vector.tensor_scalar_add(
                    kts[2][64:128, sl], pp[64:128, :], bqks[64:128, 2:3])
                repl(qts[2][64:128, sl], qts[2][0:64, sl], sl)
                # k2 lo half is needed even without PAIR (k2 is used at
                # rows 0:64 in the unpaired layout too -> always DMA)
                nc.gpsimd.dma_start(kts[2][0:64, sl], kts[2][64:128, sl])

        def proj_v():
            # V in [s, d] layout; the 65-col stride keeps the ones column
            for sb in range(SB):
                pv = shtile(f"pv{sb}")
                pvv = pv[:, 0:NH_LOC * HD]
                for dcc in range(DC):
                    nc.tensor.matmul(
                        pvv,
                        lhsT=xth(dcc, sb * 128, 128),
                        rhs=wvs[dcc][:],
                        start=(dcc == 0),
                        stop=(dcc == DC - 1),
                    )
                for h in range(NH_LOC):
                    nc.vector.tensor_copy(
                        v1s[h][:, sb * 65: sb * 65 + 64],
                        pv[:, h * HD:(h + 1) * HD],
                    )

        if level < 2:
            for sc in range(S // 512):
                proj_chunk(0, sc)
                proj_chunk(1, sc)
                proj_chunk(2, sc)
            proj_v()
            for sb in range(SB):
                ost = outst_pool.tile([128, D_MODEL], F32, tag="ost",
                                      name=f"ost{sb}")
                nc.vector.memset(ost[:], 0.0)
                nc.sync.dma_start(out_d[sb * 128:(sb + 1) * 128, :], ost[:])
            return nc

        # --- phase 2+3: attention, heads interleaved per q-chunk; the
        # final projection for a chunk's s-blocks is emitted right after its
        # three heads finish, so PE always has fill work and there is no
        # serial projection tail.
        groups = []
        j0 = 0
        while j0 < KB:
            groups.append((j0, min(GRP, KB - j0)))
            j0 += GRP

        def phase_a(h, qc, g0, glen):
            qt, kt = qts[h], kts[h]
            st = ps_st.tile([128, GRP * CHUNK], F32, tag="st",
                            name=f"st{h}_{qc}_{g0}")
            for t in range(glen):
                j = g0 + t
                if PAIR:
                    # K=64: even k-blocks read the lo partition half at PE
                    # rows 0:64, odd ones the replicated hi half at rows
                    # 64:128 -> adjacent matmuls overlap in the array
                    r = 64 * (j % 2)
                    nc.tensor.matmul(
                        st[:, t * CHUNK:(t + 1) * CHUNK],
                        lhsT=kt[r:r + 64, j * 128:(j + 1) * 128],
                        rhs=qt[r:r + 64, qc * CHUNK:(qc + 1) * CHUNK],
                        start=True,
                        stop=True,
                        tile_position=(r, 0),
                    )
                else:
                    nc.tensor.matmul(
                        st[:, t * CHUNK:(t + 1) * CHUNK],
                        lhsT=kt[:, j * 128:(j + 1) * 128],
                        rhs=qt[:, qc * CHUNK:(qc + 1) * CHUNK],
                        start=True,
                        stop=True,
                    )
            pt = pt_pool.tile([128, GRP * CHUNK], BF16, tag="pt",
                              name=f"pt{h}_{qc}_{g0}")
            nc.scalar.activation(
                pt[:, 0:glen * CHUNK],
                st[:, 0:glen * CHUNK],
                AF.Exp,
                scale=0.125,
            )
            return pt

        def fin(qc):
            # final projection for chunk qc's s-blocks (emitted one chunk
            # late so the normalize DMA round trip is off the critical path).
            # NOTE: all accumulating matmuls in one psum group must share one
            # tile_position (mixed row offsets crash the device), so every
            # head's AT / Wo tile lives at partition offset 0 with the K=128
            # zero padding.
            for sb in range(qc * (CHUNK // 128), (qc + 1) * (CHUNK // 128)):
                ost = outst_pool.tile([128, D_MODEL], F32, tag="ost",
                                      name=f"ost{sb}")
                for (n0, n1) in ((0, 512), (512, D_MODEL)):
                    po = shtile(f"fp{sb}_{n0}")
                    pon = po[:, 0:n1 - n0]
                    sb_in = sb % (CHUNK // 128)
                    for h in range(NH_LOC):
                        nc.tensor.matmul(
                            pon,
                            lhsT=ats[h][qc][:, sb_in * 128:(sb_in + 1) * 128],
                            rhs=wos[h][:, n0:n1],
                            start=(h == 0),
                            stop=(h == NH_LOC - 1),
                        )
                    nc.vector.tensor_copy(ost[:, n0:n1], pon)
                nc.gpsimd.dma_start(out_d[sb * 128:(sb + 1) * 128, :], ost[:])

        for sc in range(S // 512):
            proj_chunk(0, sc)
            proj_chunk(1, sc)
        # hoist (qc=0, h=0) scores+exp ahead of the V projection: ACT starts
        # as soon as q0/k0 exist; the matching P@V accumulation (which needs
        # V) is emitted in the main loop below and the scheduler orders it
        # after proj_v via the v1s dependency.
        pts0 = [phase_a(0, 0, g0, glen) for (g0, glen) in groups]
        proj_v()

        for qc in range(NCH):
            for h in range(NH_LOC):
                if qc == 0 and h == 2:
                    for sc in range(S // 512):
                        proj_chunk(2, sc)
                if level >= 3 and qc > 0 and h == 1:
                    fin(qc - 1)
                # acc[d, q] = sum_k [V|1][k,:]^T exp(ST)[k, q]:
                # rows 0..63 = attn_out^T (unnormalized), row 64 = sum(exp)
                acc = shtile(f"acc{h}_{qc}")
                for gi, (g0, glen) in enumerate(groups):
                    if qc == 0 and h == 0:
                        pt = pts0[gi]
                    else:
                        pt = phase_a(h, qc, g0, glen)
                    for t in range(glen):
                        j = g0 + t
                        nc.tensor.matmul(
                            acc[0:65, :],
                            lhsT=v1s[h][:, j * 65:(j + 1) * 65],
                            rhs=pt[:, t * CHUNK:(t + 1) * CHUNK],
                            start=(j == 0),
                            stop=(j == KB - 1),
                        )
                # normalize: copy the accumulator off psum (frees the shared
                # slot), broadcast the sum row to partitions 0..63 via a DRAM
                # round trip (DMA can't read PSUM; SBUF APs can't have a zero
                # partition step; DVE can't shift partitions).
                tmp = small_pool.tile([65, CHUNK], F32, tag="r1",
                                      name=f"r1_{h}_{qc}")
                nc.vector.tensor_copy(tmp[:], acc[0:65, :])
                drs = dram_pool.tile([1, CHUNK], F32, tag="drs",
                                     name=f"drs{h}_{qc}")
                nc.sync.dma_start(drs[:], tmp[64:65, :])
                rbs = rb_pool.tile([HD, CHUNK], F32, tag="rbs",
                                   name=f"rbs{h}_{qc}")
                nc.sync.dma_start(rbs[:], drs[:].to_broadcast([HD, CHUNK]))
                rbr = rb_pool.tile([HD, CHUNK], F32, tag="rbr",
                                   name=f"rbr{h}_{qc}")
                nc.vector.reciprocal_approx_fast(rbr[:], rbs[:])
                nc.vector.tensor_mul(
                    ats[h][qc][0:HD, :],
                    tmp[0:HD, :],
                    rbr[:],
                )

        if level < 3:
            for sb in range(SB):
                ost = outst_pool.tile([128, D_MODEL], F32, tag="ost",
                                      name=f"ost{sb}")
                nc.vector.memset(ost[:], 0.0)
                nc.sync.dma_start(out_d[sb * 128:(sb + 1) * 128, :], ost[:])
            return nc
        fin(NCH - 1)

    return nc


def make_nc(S=4096, level=3):
    nc = bacc.Bacc(None, target_bir_lowering=False, debug=False)
    build(nc, S, level=level)
    nc.compile()
    return nc


def shard_inputs(x, Wq, bq, Wk, bk, Wv, bv, Wo, bo, S):
    """Host-side packing of the 8 per-core input maps (bf16 casts included)."""
    import ml_dtypes

    bf = ml_dtypes.bfloat16
    in_maps = []
    for c in range(N_CORES):
        b = c // 4
        h0 = NH_LOC * (c % 4)
        cs, ce = h0 * HD, (h0 + NH_LOC) * HD
        xT = np.ascontiguousarray(x[b].T).astype(bf).reshape(DC, 128, S)

        def blkify(w2):  # [768, 128] -> [DC, 128, 128]
            return np.ascontiguousarray(w2).astype(bf).reshape(DC, 128, 128)

        wqk = np.stack([
            blkify(Wq[:, cs:cs + 2 * HD]),
            blkify(Wk[:, cs:cs + 2 * HD]),
            blkify(np.concatenate([Wq[:, cs + 2 * HD:ce],
                                   Wk[:, cs + 2 * HD:ce]], axis=1)),
        ])
        bqk = np.stack([
            bq[cs:cs + 2 * HD],
            bk[cs:cs + 2 * HD],
            np.concatenate([bq[cs + 2 * HD:ce], bk[cs + 2 * HD:ce]]),
        ], axis=1).astype(np.float32)  # [128, 3]
        wv = np.ascontiguousarray(Wv[:, cs:ce]).astype(bf).reshape(
            DC, 128, NH_LOC * HD)
        wo = np.zeros((NH_LOC, 128, D_MODEL), np.float32)
        wo[:, 0:HD, :] = Wo[cs:ce, :].reshape(NH_LOC, HD, D_MODEL)
        wo = wo.astype(bf)
        in_maps.append({"xT": xT, "wqk": wqk, "bqk": bqk, "wv": wv, "wo": wo})
    return in_maps


_NC_CACHE = {}


def kernel(x, Wq, bq, Wk, bk, Wv, bv, Wo, bo):
    from concourse import bass_utils

    x = np.asarray(x, np.float32)
    Wq, bq = np.asarray(Wq, np.float32), np.asarray(bq, np.float32)
    Wk, bk = np.asarray(Wk, np.float32), np.asarray(bk, np.float32)
    Wv, bv = np.asarray(Wv, np.float32), np.asarray(bv, np.float32)
    Wo, bo = np.asarray(Wo, np.float32), np.asarray(bo, np.float32)
    B, S, D = x.shape
    assert (B, D) == (2, D_MODEL)
    if S not in _NC_CACHE:
        _NC_CACHE[S] = make_nc(S)
    nc = _NC_CACHE[S]

    in_maps = shard_inputs(x, Wq, bq, Wk, bk, Wv, bv, Wo, bo, S)
    res = bass_utils.run_bass_kernel_spmd(nc, in_maps, core_ids=list(range(N_CORES)))

    # host reduction: sum head-group partials per batch, add bias terms
    bias = (bo.astype(np.float32)
            + bv.astype(np.float32) @ Wo.astype(np.float32))  # [768]
    out = np.empty((B, S, D_MODEL), np.float32)
    for b in range(B):
        acc = res.results[4 * b]["out"].astype(np.float32).copy()
        for c in range(4 * b + 1, 4 * b + 4):
            acc += res.results[c]["out"]
        out[b] = acc + bias
    return out


# revision 3
# speedup vs baseline: 1.0586x; 1.0226x over previous
"""Multi-head attention (B=2, S=4096, D=768, H=12, hd=64) on 8 trn2 NeuronCores.

Sharding: core c -> batch b = c//4, heads [3*(c%4), 3*(c%4)+3)  (batch- and
head-parallel; no device collectives).  Each core computes the partial
output  sum_h softmax((x Wq_h + bq_h)(x Wk_h + bk_h)^T / 8) (x Wv_h) Wo_h
for its 3 heads as a full [S, 768] tensor (bf16); the host sums the 4
partials per batch in f32 and adds the bias terms (bo + bv @ Wo, since
softmax rows sum to 1).

Per-core device algorithm (all matmuls bf16, f32 psum accumulate).  The
engines execute their instruction queues strictly in order, and ACT (the
exp engine, ~1 elem/lane/cycle on 3*S*S scores) is the critical resource
at ~390us busy -- so the emission order below is built to keep ACT fed:

  - host ships x[b]^T chunk-major as [S/512, 128, 6*512] so each 512-col
    chunk of all 6 d-blocks is ONE big DMA; weights are packed for
    single-DMA loads as well
  - q/k projections emitted per chunk (blk0 = [q0 q1], blk1 = [k0 k1]),
    interleaved with the hoisted (h0, qc0) score groups so the first exp
    lands within a few us; each head's 64 d-rows are DMA-replicated to
    the other partition half (only DMA can shift partitions)
  - scores computed transposed as K=64 matmuls: ST[k-block, q-chunk] =
    kT^T q with tile_position=(0,0) for even k-blocks (lo half) and
    (64,0) for odd k-blocks (hi half); adjacent matmuls hit different PE
    row-groups and execute concurrently (measured dt_start ~3ns),
    recovering the utilization lost to hd=64 < 128.  exp on ACT (no max
    subtraction: |scores/8| <~ 2 for this problem)
  - attn_out^T accumulated directly: acc[0:64, q] = sum_k V[k,:]^T P^T[k, q],
    row 64 = sum_k exp (the ones column); no output transpose is needed
  - blk2 ([q2 k2]) projections and the deferred final projections are
    emitted in 2 half-batches at the h1/h2 loop heads so no single PE
    FIFO block exceeds what ACT has queued
  - normalized with reciprocal_approx_fast of row 64 (~5x faster than
    exact; denominators are O(1e3) so approx is safe), partition-
    broadcast via a DRAM round trip, then the final projection against
    Wo rows (K=128 with zero padding: mixed tile_position row offsets in
    one accumulation group crash the device)
"""

import numpy as np
from contextlib import ExitStack

import concourse.bass as bass
import concourse.bacc as bacc
import concourse.mybir as mybir
from concourse import tile

BF16 = mybir.dt.bfloat16
F32 = mybir.dt.float32
AF = mybir.ActivationFunctionType

D_MODEL = 768
N_HEADS = 12
HD = 64
N_CORES = 8
NH_LOC = 3          # heads per core
DC = D_MODEL // 128  # 6 chunks of d_model
CHUNK = 512          # q columns processed per score chunk
GRP = 3              # k-blocks (of 128) per psum score tile / exp call
PAIR = True          # K=64 row-tiled score matmuls (vs K=128 zero-padded)


def build(nc, S, level=3):
    """Emit the per-core program (SPMD; all cores run this with their shard).

    level: debug knob — 1 = projections only, 2 = + attention, 3 = full.
    """
    SB = S // 128     # seq blocks of 128
    NCH = S // CHUNK  # q chunks
    KB = S // 128     # k blocks of 128
    NXH = S // 512    # x column chunks

    xT_d = nc.declare_dram_parameter("xT", [NXH, 128, DC * 512], BF16,
                                     isOutput=False)
    wqk_d = nc.declare_dram_parameter("wqk", [3, 128, DC * 128], BF16,
                                      isOutput=False)
    bqk_d = nc.declare_dram_parameter("bqk", [128, 3], F32, isOutput=False)
    wv_d = nc.declare_dram_parameter("wv", [128, DC * NH_LOC * HD], BF16,
                                     isOutput=False)
    wo_d = nc.declare_dram_parameter("wo", [128, NH_LOC * D_MODEL], BF16,
                                     isOutput=False)
    out_d = nc.declare_dram_parameter("out", [S, D_MODEL], BF16, isOutput=True)

    with tile.TileContext(nc) as tc, ExitStack() as ctx:
        const = ctx.enter_context(tc.tile_pool(name="const", bufs=1))

        def ctile(name, shape, dt):
            return const.tile(shape, dt, tag=name, name=name)

        # --- constants / long-lived tensors -------------------------------
        xts = [ctile(f"xt{c}", [128, DC * 512], BF16) for c in range(NXH)]

        def xth(dcc, off, ln):
            # [off, off+ln) of logical xT d-block dcc (ln within one chunk)
            t = xts[off // 512]
            lo = off % 512
            return t[:, dcc * 512 + lo: dcc * 512 + lo + ln]
        wqks = [ctile(f"wqk{i}", [128, DC * 128], BF16) for i in range(3)]
        bqks = ctile("bqk", [128, 3], F32)
        wvs = ctile("wv", [128, DC * NH_LOC * HD], BF16)
        wos = ctile("wo", [128, NH_LOC * D_MODEL], BF16)
        v1s = [ctile(f"v1_{h}", [128, 65 * KB], BF16) for h in range(NH_LOC)]
        qts = [ctile(f"qt{i}", [128, S], BF16) for i in range(NH_LOC)]
        kts = [ctile(f"kt{i}", [128, S], BF16) for i in range(NH_LOC)]
        ats = [[ctile(f"at{i}_{qc}", [128, CHUNK], BF16)
                for qc in range(NCH)] for i in range(NH_LOC)]

        pt_pool = ctx.enter_context(tc.tile_pool(name="pt", bufs=12))
        outst_pool = ctx.enter_context(tc.tile_pool(name="outst", bufs=2))
        small_pool = ctx.enter_context(tc.tile_pool(name="small", bufs=2))
        rb_pool = ctx.enter_context(tc.tile_pool(name="rb", bufs=2))
        dram_pool = ctx.enter_context(tc.tile_pool(name="drs", bufs=3, space="DRAM"))
        # ONE psum pool layout for the whole kernel (no pool releases -> no
        # cross-phase serialization): 6 banks of score tiles + 2 banks shared
        # (same tag) by projection / P@V-accumulator / final-projection tiles.
        ps_st = ctx.enter_context(tc.tile_pool(name="ps_st", bufs=2, space="PSUM"))
        ps_sh = ctx.enter_context(tc.tile_pool(name="ps_sh", bufs=2, space="PSUM"))

        def shtile(nm):
            return ps_sh.tile([128, 512], F32, tag="ps", name=nm)

        # --- load inputs ---------------------------------------------------
        # q/k weights + x chunk 0 first: the first projection depends only
        # on these, so the PE starts within a few us
        nc.sync.dma_start(wqks[0][:], wqk_d[0])
        nc.sync.dma_start(xts[0][:], xT_d[0])
        nc.sync.dma_start(wqks[1][:], wqk_d[1])
        nc.sync.dma_start(bqks[:], bqk_d[:])
        for ch in range(1, 3):
            nc.sync.dma_start(xts[ch][:], xT_d[ch])
        nc.sync.dma_start(wqks[2][:], wqk_d[2])
        nc.sync.dma_start(wvs[:], wv_d[:])
        for ch in range(3, NXH):
            nc.sync.dma_start(xts[ch][:], xT_d[ch])
        nc.sync.dma_start(wos[:], wo_d[:])

        # --- phase 1: projections -----------------------------------------
        def repl(dst, lo_src):
            # replicate a 64-partition half to the other half (DMA only)
            if PAIR:
                nc.gpsimd.dma_start(dst, lo_src)

        def proj_chunk(blk, sc):
            # qT / kT block: [d_out(128 part), s] = W_blk^T x^T
            # blk0 = [q0 q1] -> Q0 rows 0:64 / Q1 rows 64:128
            # blk1 = [k0 k1] -> K0 / K1
            # blk2 = [q2 k2] -> Q2 rows 0:64; k2 rows 64:128 (bias-added in
            #   place, partitions match the psum half)
            # each 64-row half is then DMA-replicated to the other half
            pp = shtile(f"pp{blk}_{sc}")
            for dcc in range(DC):
                nc.tensor.matmul(
                    pp[:],
                    lhsT=wqks[blk][:, dcc * 128:(dcc + 1) * 128],
                    rhs=xth(dcc, sc * 512, 512),
                    start=(dcc == 0),
                    stop=(dcc == DC - 1),
                )
            sl = slice(sc * 512, (sc + 1) * 512)
            if blk == 0 or blk == 1:
                dsts = qts if blk == 0 else kts
                nc.vector.tensor_scalar_add(
                    dsts[0][0:64, sl], pp[0:64, :], bqks[0:64, blk:blk + 1])
                nc.vector.tensor_scalar_add(
                    dsts[1][64:128, sl], pp[64:128, :], bqks[64:128, blk:blk + 1])
                repl(dsts[0][64:128, sl], dsts[0][0:64, sl])
                repl(dsts[1][0:64, sl], dsts[1][64:128, sl])
            else:
                nc.vector.tensor_scalar_add(
                    qts[2][0:64, sl], pp[0:64, :], bqks[0:64, 2:3])
                nc.vector.tensor_scalar_add(
                    kts[2][64:128, sl], pp[64:128, :], bqks[64:128, 2:3])
                repl(qts[2][64:128, sl], qts[2][0:64, sl])
                # k2 lo half is needed even without PAIR
                nc.gpsimd.dma_start(kts[2][0:64, sl], kts[2][64:128, sl])

        def proj_v_unit(sb):
            # V in [s, d] layout; the 65-col stride keeps the ones column
            pv = shtile(f"pv{sb}")
            pvv = pv[:, 0:NH_LOC * HD]
            for dcc in range(DC):
                nc.tensor.matmul(
                    pvv,
                    lhsT=xth(dcc, sb * 128, 128),
                    rhs=wvs[:, (dcc * NH_LOC) * HD:(dcc * NH_LOC + NH_LOC) * HD],
                    start=(dcc == 0),
                    stop=(dcc == DC - 1),
                )
            for h in range(NH_LOC):
                nc.vector.tensor_copy(
                    v1s[h][:, sb * 65: sb * 65 + 64],
                    pv[:, h * HD:(h + 1) * HD],
                )

        if level < 2:
            for sc in range(NXH):
                proj_chunk(0, sc)
                proj_chunk(1, sc)
                proj_chunk(2, sc)
            for h in range(NH_LOC):
                nc.vector.memset(v1s[h][:], 1.0)
            for sb in range(SB):
                proj_v_unit(sb)
            for sb in range(SB):
                ost = outst_pool.tile([128, D_MODEL], BF16, tag="ost",
                                      name=f"ost{sb}")
                nc.vector.memset(ost[:], 0.0)
                nc.sync.dma_start(out_d[sb * 128:(sb + 1) * 128, :], ost[:])
            return nc

        # --- phase 2+3: attention -----------------------------------------
        groups = []
        j0 = 0
        while j0 < KB:
            groups.append((j0, min(GRP, KB - j0)))
            j0 += GRP
        NG = len(groups)

        def phase_a(h, qc, g0, glen):
            qt, kt = qts[h], kts[h]
            st = ps_st.tile([128, GRP * CHUNK], F32, tag="st",
                            name=f"st{h}_{qc}_{g0}")
            for t in range(glen):
                j = g0 + t
                if PAIR:
                    # K=64: even k-blocks read the lo partition half at PE
                    # rows 0:64, odd ones the replicated hi half at rows
                    # 64:128 -> adjacent matmuls overlap in the array
                    r = 64 * (j % 2)
                    nc.tensor.matmul(
                        st[:, t * CHUNK:(t + 1) * CHUNK],
                        lhsT=kt[r:r + 64, j * 128:(j + 1) * 128],
                        rhs=qt[r:r + 64, qc * CHUNK:(qc + 1) * CHUNK],
                        start=True,
                        stop=True,
                        tile_position=(r, 0),
                    )
                else:
                    nc.tensor.matmul(
                        st[:, t * CHUNK:(t + 1) * CHUNK],
                        lhsT=kt[:, j * 128:(j + 1) * 128],
                        rhs=qt[:, qc * CHUNK:(qc + 1) * CHUNK],
                        start=True,
                        stop=True,
                    )
            pt = pt_pool.tile([128, GRP * CHUNK], BF16, tag="pt",
                              name=f"pt{h}_{qc}_{g0}")
            nc.scalar.activation(
                pt[:, 0:glen * CHUNK],
                st[:, 0:glen * CHUNK],
                AF.Exp,
                scale=0.125,
            )
            return pt

        def fin_unit(qc, sb):
            # final projection for one 128-row s-block of chunk qc (deferred
            # so the normalize round trip is off the critical path).
            # NOTE: all accumulating matmuls in one psum group must share one
            # tile_position (mixed row offsets crash the device), so every
            # head's AT / Wo tile lives at partition offset 0 with the K=128
            # zero padding.
            ost = outst_pool.tile([128, D_MODEL], BF16, tag="ost",
                                  name=f"ost{sb}")
            for (n0, n1) in ((0, 512), (512, D_MODEL)):
                po = shtile(f"fp{sb}_{n0}")
                pon = po[:, 0:n1 - n0]
                sb_in = sb % (CHUNK // 128)
                for h in range(NH_LOC):
                    nc.tensor.matmul(
                        pon,
                        lhsT=ats[h][qc][:, sb_in * 128:(sb_in + 1) * 128],
                        rhs=wos[:, h * D_MODEL + n0:h * D_MODEL + n1],
                        start=(h == 0),
                        stop=(h == NH_LOC - 1),
                    )
                nc.vector.tensor_copy(ost[:, n0:n1], pon)
            nc.gpsimd.dma_start(out_d[sb * 128:(sb + 1) * 128, :], ost[:])

        # interleave q/k projection chunks with the hoisted (h0, qc0) score
        # groups: group g touches k-blocks 3g..3g+2 -> ready after chunk
        # (3g+2)//4 (and q-chunk 0).  ACT starts exp'ing within a few us.
        hoist_after = {}  # chunk -> list of group indices
        for gi, (g0, glen) in enumerate(groups):
            need = (g0 + glen - 1) // 4
            hoist_after.setdefault(need, []).append(gi)
        pts0 = [None] * NG
        for sc in range(NXH):
            proj_chunk(0, sc)
            proj_chunk(1, sc)
            for gi in hoist_after.get(sc, []):
                g0, glen = groups[gi]
                pts0[gi] = phase_a(0, 0, g0, glen)
        # ones columns (DVE: keeps gpsimd free for the replication DMAs)
        for h in range(NH_LOC):
            nc.vector.memset(v1s[h][:], 1.0)
        for sb in range(SB):
            proj_v_unit(sb)
        # fin zero padding (needed first by fin(qc0) much later)
        for h in range(NH_LOC):
            for qc in range(NCH):
                nc.vector.memset(ats[h][qc][HD:128, :], 0.0)

        for qc in range(NCH):
            for h in range(NH_LOC):
                # deferred PE batches at loop heads, split h1/h2 so no FIFO
                # block exceeds ACT's queued work
                if qc == 0 and h == 1:
                    for sc in range(0, NXH // 2):
                        proj_chunk(2, sc)
                if qc == 0 and h == 2:
                    for sc in range(NXH // 2, NXH):
                        proj_chunk(2, sc)
                if level >= 3 and qc > 0:
                    sbs = range(qc * (CHUNK // 128) - 4, qc * (CHUNK // 128))
                    if h == 1:
                        for sb in sbs[:2]:
                            fin_unit(qc - 1, sb)
                    elif h == 2:
                        for sb in sbs[2:]:
                            fin_unit(qc - 1, sb)
                # acc[d, q] = sum_k [V|1][k,:]^T exp(ST)[k, q]:
                # rows 0..63 = attn_out^T (unnormalized), row 64 = sum(exp)
                acc = shtile(f"acc{h}_{qc}")
                for gi, (g0, glen) in enumerate(groups):
                    if qc == 0 and h == 0:
                        pt = pts0[gi]
                    else:
                        pt = phase_a(h, qc, g0, glen)
                    for t in range(glen):
                        j = g0 + t
                        nc.tensor.matmul(
                            acc[0:65, :],
                            lhsT=v1s[h][:, j * 65:(j + 1) * 65],
                            rhs=pt[:, t * CHUNK:(t + 1) * CHUNK],
                            start=(j == 0),
                            stop=(j == KB - 1),
                        )
                # normalize: copy the accumulator off psum (frees the shared
                # slot), broadcast the sum row to partitions 0..63 via a DRAM
                # round trip (DMA can't read PSUM; SBUF APs can't have a zero
                # partition step; DVE can't shift partitions).
                tmp = small_pool.tile([65, CHUNK], F32, tag="r1",
                                      name=f"r1_{h}_{qc}")
                nc.vector.tensor_copy(tmp[:], acc[0:65, :])
                drs = dram_pool.tile([1, CHUNK], F32, tag="drs",
                                     name=f"drs{h}_{qc}")
                nc.sync.dma_start(drs[:], tmp[64:65, :])
                rbs = rb_pool.tile([HD, CHUNK], F32, tag="rbs",
                                   name=f"rbs{h}_{qc}")
                nc.sync.dma_start(rbs[:], drs[:].to_broadcast([HD, CHUNK]))
                rbr = rb_pool.tile([HD, CHUNK], F32, tag="rbr",
                                   name=f"rbr{h}_{qc}")
                nc.vector.reciprocal_approx_fast(rbr[:], rbs[:])
                nc.vector.tensor_mul(
                    ats[h][qc][0:HD, :],
                    tmp[0:HD, :],
                    rbr[:],
                )

        if level < 3:
            for sb in range(SB):
                ost = outst_pool.tile([128, D_MODEL], BF16, tag="ost",
                                      name=f"ost{sb}")
                nc.vector.memset(ost[:], 0.0)
                nc.sync.dma_start(out_d[sb * 128:(sb + 1) * 128, :], ost[:])
            return nc
        for sb in range((NCH - 1) * (CHUNK // 128), NCH * (CHUNK // 128)):
            fin_unit(NCH - 1, sb)

    return nc


def make_nc(S=4096, level=3):
    nc = bacc.Bacc(None, target_bir_lowering=False, debug=False)
    build(nc, S, level=level)
    nc.compile()
    return nc


def shard_inputs(x, Wq, bq, Wk, bk, Wv, bv, Wo, bo, S):
    """Host-side packing of the 8 per-core input maps (bf16 casts included)."""
    import ml_dtypes

    bf = ml_dtypes.bfloat16
    NXH = S // 512
    in_maps = []
    for c in range(N_CORES):
        b = c // 4
        h0 = NH_LOC * (c % 4)
        cs, ce = h0 * HD, (h0 + NH_LOC) * HD
        # chunk-major xT: [NXH, 128, DC*512]; chunk ch holds columns
        # [ch*512,(ch+1)*512) of all DC d-blocks side by side
        xt = np.ascontiguousarray(x[b].T).astype(bf).reshape(DC, 128, S)
        xT = np.ascontiguousarray(
            xt.reshape(DC, 128, NXH, 512).transpose(2, 1, 0, 3)
        ).reshape(NXH, 128, DC * 512)

        def pack_w(w2):  # [768, n] -> [128, DC*n] (d-blocks side by side)
            n = w2.shape[1]
            return np.ascontiguousarray(
                w2.reshape(DC, 128, n).transpose(1, 0, 2)
            ).astype(bf).reshape(128, DC * n)

        wqk = np.stack([
            pack_w(Wq[:, cs:cs + 2 * HD]),
            pack_w(Wk[:, cs:cs + 2 * HD]),
            pack_w(np.concatenate([Wq[:, cs + 2 * HD:ce],
                                   Wk[:, cs + 2 * HD:ce]], axis=1)),
        ])
        bqk = np.stack([
            bq[cs:cs + 2 * HD],
            bk[cs:cs + 2 * HD],
            np.concatenate([bq[cs + 2 * HD:ce], bk[cs + 2 * HD:ce]]),
        ], axis=1).astype(np.float32)  # [128, 3]
        wv = pack_w(Wv[:, cs:ce])
        wo = np.zeros((NH_LOC, 128, D_MODEL), np.float32)
        wo[:, 0:HD, :] = Wo[cs:ce, :].reshape(NH_LOC, HD, D_MODEL)
        wo = np.ascontiguousarray(wo.transpose(1, 0, 2)).astype(bf).reshape(
            128, NH_LOC * D_MODEL)
        in_maps.append({"xT": xT, "wqk": wqk, "bqk": bqk, "wv": wv, "wo": wo})
    return in_maps


_NC_CACHE = {}


def kernel(x, Wq, bq, Wk, bk, Wv, bv, Wo, bo):
    from concourse import bass_utils

    x = np.asarray(x, np.float32)
    Wq, bq = np.asarray(Wq, np.float32), np.asarray(bq, np.float32)
    Wk, bk = np.asarray(Wk, np.float32), np.asarray(bk, np.float32)
    Wv, bv = np.asarray(Wv, np.float32), np.asarray(bv, np.float32)
    Wo, bo = np.asarray(Wo, np.float32), np.asarray(bo, np.float32)
    B, S, D = x.shape
    assert (B, D) == (2, D_MODEL)
    if S not in _NC_CACHE:
        _NC_CACHE[S] = make_nc(S)
    nc = _NC_CACHE[S]

    in_maps = shard_inputs(x, Wq, bq, Wk, bk, Wv, bv, Wo, bo, S)
    res = bass_utils.run_bass_kernel_spmd(nc, in_maps, core_ids=list(range(N_CORES)))

    # host reduction: sum head-group partials per batch, add bias terms
    bias = (bo.astype(np.float32)
            + bv.astype(np.float32) @ Wo.astype(np.float32))  # [768]
    out = np.empty((B, S, D_MODEL), np.float32)
    for b in range(B):
        acc = res.results[4 * b]["out"].astype(np.float32)
        for c in range(4 * b + 1, 4 * b + 4):
            acc += res.results[c]["out"].astype(np.float32)
        out[b] = acc + bias
    return out
